# revision 2
# baseline (speedup 1.0000x reference)
"""Distributed ARMAConv kernel for 8 TRN2 NeuronCores (Bass/Tile) — Design G.

Math: ARMAConv with identical stacks and softmax weights summing to 1 equals a
single stack: two layers of  current = 0.9 * (D^-1/2 A D^-1/2) @ current + 0.1*x.
Folding dis = deg^-1/2:  dis*h1 = 0.9*dis^2*agg + 0.1*dis*x  (layer-1 epilogue
produces the layer-2 gather table directly);  out = 0.9*dis*agg2 + 0.1*x.

Per layer, per core (dst rows sharded 12544/core):
  - edges sorted (section, bank, chunk); dma_gather fetches each edge's
    (dis-prescaled) source row EDGE-MAJOR into tmp groups of 128 slots.
    No scatter, no ELL rectangles: desc i -> (partition i%128, group i//128).
  - PE routes slots->lanes: psum[lane, ch] += onehot[slot,lane].T @ tmp[slot,ch]
    accumulated per dst chunk. One-hots are 0/1 fp32 generated in bulk on DVE
    via broadcast is_equal against an iota row; padding/out-of-chunk slots get
    sentinel lane 200 (never matches) so garbage slots route nowhere.
  - DVE epilogue per section: prop = ps*s1 + xs0 (layer1), out = ps*s3 + xl.
  - AllGather (2 slices, aligned to 2 source-bank windows each) replicates the
    layer-1 output as the layer-2 gather table.

SPMD: one program runs on all 8 cores, so group counts per (section, bank) and
the matmul column schedule are maxed/unioned across cores; per-core lane tables
fill the variable part. Host work: O(E) sorting/relabeling + index/lane-table
packing and the dis*x input scaling, as in the baseline.
"""
import sys
if '/opt/trn_rl_repo' not in sys.path:
    sys.path.insert(0, '/opt/trn_rl_repo')
import numpy as np

from concourse import bass, mybir, bacc
import concourse.tile as tile
from concourse import bass_utils

F32 = mybir.dt.float32
F16 = mybir.dt.float16
I16 = mybir.dt.int16
P = 128
ROWP = 128           # fp16 table row pitch in elements (256B); data in [0:64]


def _dma_gather_small(g, out_ap, in_ap, idxs_ap, num_idxs, elem_size,
                      elem_step, queue_num):
    """nc.gpsimd.dma_gather clone for sub-256B fetches: elem_size (fetched
    elements) may be 128B while elem_step (row pitch) stays a 256B multiple.
    Mirrors bass.BassGpSimd.dma_gather's non-transpose HBM path."""
    from concourse.bass import MemorySpace
    from concourse import ap_utils
    g._assert_queue_num(queue_num)
    assert idxs_ap.dtype == mybir.dt.int16
    assert in_ap.dtype == out_ap.dtype
    assert in_ap.space == MemorySpace.DRAM
    assert idxs_ap.space == MemorySpace.SBUF and out_ap.space == MemorySpace.SBUF
    assert ap_utils.ap_is_contiguous(out_ap.ap[1:])
    assert ap_utils.ap_is_contiguous(idxs_ap.ap[1:])
    assert in_ap.ap[-1][1] == elem_size and out_ap.ap[-1][1] == elem_size
    assert out_ap.ap[0][1] * out_ap.ap[1][1] == num_idxs
    assert in_ap.ap[0][0] == elem_step
    stride_bytes = elem_step * mybir.dt.size(in_ap.dtype)
    stride_bytes_256 = stride_bytes // 256
    assert stride_bytes_256 * 256 == stride_bytes and stride_bytes_256 < 256
    inst = g.add_instruction(
        mybir.InstDMAGatherAnt(
            name=g.bass.get_next_instruction_name(),
            ins=[*g.lower_ap_dma(in_ap, for_custom_bir_dma=True),
                 g.lower_ap(idxs_ap),
                 g.lower_val_access(g.to_reg(num_idxs))],
            outs=[g.lower_ap(out_ap)],
            transpose=False, num_idxs=num_idxs, elem_size=elem_size,
            stride_bytes_256=stride_bytes_256, gen_mode=0,
            single_packet=True, queue_num=queue_num,
            sbuf_tokens_per_rank=0, sbuf_free_dim_per_rank=0,
            sbuf_free_dim_pad_per_rank=0, sbuf_byte_offset=0))
    return inst

ALPHA = 0.1
PROP_SCALE = 0.9
SENTINEL = 200.0
MAXN = 1024          # descriptors per dma_gather (HW single-packet cap)
WBLK = 32            # one-hot columns generated per DVE op / W buffer


class Cfg:
    def __init__(self, n_nodes, channels, n_cores, nch, k_chunks, n_banks):
        self.N_NODES = n_nodes
        self.C = channels
        self.N_CORES = n_cores
        self.NCH = nch                      # chunks per core
        self.LOCAL = nch * P
        self.N_PAD = n_cores * self.LOCAL
        self.K = k_chunks                   # chunks per section
        assert nch % k_chunks == 0
        self.NSEC = nch // k_chunks
        self.N_BANKS = n_banks
        assert self.N_PAD % n_banks == 0
        self.BANK = self.N_PAD // n_banks
        assert self.BANK <= 32768
        # AG split: slice A = chunks [0, CH0), must cover banks 0..N_BANKS/2-1
        self.CH0 = nch // 2
        assert (self.CH0 * P * n_cores) == self.BANK * (n_banks // 2)
        assert self.CH0 % k_chunks == 0
        self.SPLIT_SEC = self.CH0 // k_chunks - 1   # AG-A after this section


CFG_FULL = Cfg(n_nodes=100000, channels=64, n_cores=8, nch=98, k_chunks=7,
               n_banks=4)


def _gpos(cfg, node):
    """Table position of each node: slice-major, core-major, lane-major."""
    core = node // cfg.LOCAL
    r = node % cfg.LOCAL
    ch, lane = r // P, r % P
    half = (ch >= cfg.CH0).astype(np.int64)
    k0 = cfg.CH0
    k1 = cfg.NCH - cfg.CH0
    kk = np.where(half == 0, k0, k1)
    chh = ch - half * cfg.CH0
    base = half * (cfg.N_CORES * k0 * P)
    return base + core * (kk * P) + lane * kk + chh


def _preprocess(cfg, edge_index):
    row = np.asarray(edge_index[0], np.int64)
    col = np.asarray(edge_index[1], np.int64)
    deg = np.bincount(row, minlength=cfg.N_PAD).astype(np.int64)
    dis = np.where(deg > 0, 1.0 / np.sqrt(np.maximum(deg, 1)), 0.0).astype(np.float32)

    gpos = _gpos(cfg, np.arange(cfg.N_PAD))
    gp_edge = gpos[col]
    bank_e = gp_edge // cfg.BANK
    glocal_e = gp_edge % cfg.BANK

    core_e = row // cfg.LOCAL
    r_e = row % cfg.LOCAL
    chunk_e = r_e // P
    lane_e = r_e % P
    sec_e = chunk_e // cfg.K

    # per-core streams sorted by (section, bank, chunk)
    NSEC, NB, K = cfg.NSEC, cfg.N_BANKS, cfg.K
    per_core = []
    cnt = np.zeros((cfg.N_CORES, NSEC, NB), np.int64)
    for c in range(cfg.N_CORES):
        m = core_e == c
        key = np.lexsort((chunk_e[m], bank_e[m], sec_e[m]))
        per_core.append((sec_e[m][key], bank_e[m][key], chunk_e[m][key],
                         lane_e[m][key], glocal_e[m][key]))
        np.add.at(cnt[c], (sec_e[m][key], bank_e[m][key]), 1)

    G_sb = np.maximum(1, (cnt.max(axis=0) + P - 1) // P)   # [NSEC, NB] shared
    grp_base = np.zeros((NSEC, NB), np.int64)              # group base in section
    for s in range(NSEC):
        acc = 0
        for b in range(NB):
            grp_base[s, b] = acc
            acc += G_sb[s, b]
    sec_groups = G_sb.sum(axis=1)                          # groups per section
    GMAX = int(sec_groups.max())
    TOT = int(sec_groups.sum()) * P                        # descs per layer

    # desc stream offsets (shared): per (section, bank) run start in stream
    run_off = np.zeros((NSEC, NB), np.int64)
    off = 0
    for s in range(NSEC):
        for b in range(NB):
            run_off[s, b] = off
            off += int(G_sb[s, b]) * P
    assert off == TOT

    # build gtab (idx stream) and per-(sec,b,g) chunk sets per core
    gtab = np.zeros((cfg.N_CORES, TOT), np.int64)
    slot_chunk = np.full((cfg.N_CORES, TOT), -1, np.int64)
    slot_lane = np.zeros((cfg.N_CORES, TOT), np.int64)
    for c in range(cfg.N_CORES):
        se, be, che, lae, gle = per_core[c]
        pos_in_run = np.zeros(se.size, np.int64)
        # position within (sec,bank) run: cumcount
        run_id = se * NB + be
        order_start = np.zeros(NSEC * NB + 1, np.int64)
        np.add.at(order_start[1:], run_id, 1)
        order_start = np.cumsum(order_start)
        pos_in_run = np.arange(se.size) - order_start[run_id]
        pos = run_off[se, be] + pos_in_run
        gtab[c, pos] = gle
        slot_chunk[c, pos] = che
        slot_lane[c, pos] = lae

    # matmul column schedule: per (sec, bank, group): union of chunks touched
    # across cores. Columns ordered chunk-major within each section.
    # col list entries: (sec, local_chunk, bank, group_in_run)
    cols = []           # global schedule
    for s in range(NSEC):
        by_chunk = [[] for _ in range(K)]
        for b in range(NB):
            for g in range(int(G_sb[s, b])):
                lo = run_off[s, b] + g * P
                chs = slot_chunk[:, lo:lo + P]
                touched = np.unique(chs[chs >= 0])
                if touched.size == 0:
                    # group entirely padding on all cores: still needs one
                    # column (all-sentinel) to keep schedule simple? skip it.
                    continue
                for ch in touched:
                    by_chunk[int(ch) - s * K].append((b, g))
        for lc in range(K):
            for (b, g) in by_chunk[lc]:
                cols.append((s, lc, b, g))
    NCOL = len(cols)

    # lane tables [core][P, NCOL]
    lane_tbl = np.full((cfg.N_CORES, P, NCOL), SENTINEL, np.float16)
    for ci, (s, lc, b, g) in enumerate(cols):
        lo = run_off[s, b] + g * P
        ch = s * K + lc
        for c in range(cfg.N_CORES):
            m = slot_chunk[c, lo:lo + P] == ch
            lane_tbl[c, m, ci] = slot_lane[c, lo:lo + P][m]

    # per-section column ranges + per-(section,chunk) start/stop col indices
    sec_col_lo = np.zeros(NSEC + 1, np.int64)
    for ci, (s, lc, b, g) in enumerate(cols):
        sec_col_lo[s + 1] = ci + 1
    for s in range(NSEC):
        sec_col_lo[s + 1] = max(sec_col_lo[s + 1], sec_col_lo[s])

    meta = dict(G_sb=G_sb, grp_base=grp_base, sec_groups=sec_groups, GMAX=GMAX,
                TOT=TOT, run_off=run_off, cols=cols, NCOL=NCOL,
                sec_col_lo=sec_col_lo, dis=dis, gpos=gpos)
    return meta, gtab, lane_tbl


def _wrap16(tab):
    """[cores, TOT] int idx -> [cores, P, TOT//16] int16 wrapped+replicated."""
    ncore, TOT = tab.shape
    w = tab.reshape(ncore, TOT // 16, 16).transpose(0, 2, 1)  # [c,16,T/16]
    return np.ascontiguousarray(np.tile(w, (1, 8, 1)).astype(np.int16))


def _build_program(cfg, meta, queue_map=None, compile_=True):
    C = cfg.C
    NCH, NSEC, K, NB = cfg.NCH, cfg.NSEC, cfg.K, cfg.N_BANKS
    G_sb, grp_base = meta["G_sb"], meta["grp_base"]
    sec_groups, GMAX = meta["sec_groups"], meta["GMAX"]
    TOT, run_off = meta["TOT"], meta["run_off"]
    cols, NCOL = meta["cols"], meta["NCOL"]
    CH0 = cfg.CH0

    nc = bacc.Bacc("TRN2", target_bir_lowering=False, debug=False,
                   num_devices=cfg.N_CORES, num_swdge_queues=4)
    xpc_d = nc.dram_tensor("x_pc", [P, NCH * C], F32, kind="ExternalInput")
    dis_d = nc.dram_tensor("dis_pc", [P, NCH], F32, kind="ExternalInput")
    gtab_d = nc.dram_tensor("gtab", [P, TOT // 16], I16, kind="ExternalInput")
    ltab_d = nc.dram_tensor("ltab", [P, NCOL], F16, kind="ExternalInput")
    iota_d = nc.dram_tensor("iota", [P, P], F16, kind="ExternalInput")
    xs_src_d = nc.dram_tensor("xs_src", [cfg.N_PAD, ROWP], F16, kind="ExternalInput")
    out_d = nc.dram_tensor("out", [P, NCH * C], F32, kind="ExternalOutput")

    ag_a = nc.dram_tensor("ag_a", [CH0 * P, ROWP], F16, kind="Internal")
    ag_b = nc.dram_tensor("ag_b", [(NCH - CH0) * P, ROWP], F16, kind="Internal")
    HALF_ROWS = cfg.N_CORES * CH0 * P
    xs_fullA = nc.dram_tensor("xs_fullA", [HALF_ROWS, ROWP], F16,
                              kind="Internal", addr_space="Shared")
    xs_fullB = nc.dram_tensor("xs_fullB", [cfg.N_PAD - HALF_ROWS, ROWP], F16,
                              kind="Internal", addr_space="Shared")
    RG = [list(range(cfg.N_CORES))]
    gather_insts = []

    with tile.TileContext(nc) as tc:
        with (
            tc.tile_pool(name="main", bufs=1) as mp,
            tc.tile_pool(name="tmpp", bufs=5) as tp,
            tc.tile_pool(name="wp", bufs=3) as wp,
            tc.tile_pool(name="pp", bufs=2, space="PSUM") as pp,
        ):
            dis = mp.tile([P, NCH], F32)
            s1 = mp.tile([P, NCH], F32)
            s3 = mp.tile([P, NCH], F32)
            xs0 = mp.tile([P, NCH, C], F32)
            prop = mp.tile([P, NCH, C], F32)
            prop16 = mp.tile([P, NCH, ROWP], F16)
            gtab = mp.tile([P, TOT // 16], I16)
            ltab = mp.tile([P, NCOL], F16)
            iota = mp.tile([P, P], F16)

            nc.vector.memset(prop16[:], 0.0)
            nc.sync.dma_start(dis[:], dis_d[:])
            nc.sync.dma_start(gtab[:], gtab_d[:])
            nc.sync.dma_start(ltab[:], ltab_d[:])
            nc.sync.dma_start(iota[:], iota_d[:])
            nc.sync.dma_start(xs0[:], xpc_d[:].rearrange("p (k c) -> p k c", c=C))
            # s1 = 0.9*dis^2 ; s3 = 0.9*dis ; xs0 = alpha*dis*x
            nc.vector.tensor_tensor(out=s1[:], in0=dis[:], in1=dis[:],
                                    op=mybir.AluOpType.mult)
            nc.vector.tensor_scalar_mul(s1[:], s1[:], PROP_SCALE)
            nc.vector.tensor_scalar_mul(s3[:], dis[:], PROP_SCALE)
            disb = dis[:].rearrange("p (k o) -> p k o", o=1).to_broadcast([P, NCH, C])
            nc.vector.tensor_tensor(out=xs0[:], in0=xs0[:], in1=disb,
                                    op=mybir.AluOpType.mult)
            nc.vector.tensor_scalar_mul(xs0[:], xs0[:], ALPHA)

            qn = 0
            HB = cfg.N_BANKS // 2
            LOOKAHEAD = 4

            def emit_bank(layer, s, b, tmph):
                gs = int(G_sb[s, b])
                gb = int(grp_base[s, b])
                doff = int(run_off[s, b])
                g0 = 0
                if layer == 0:
                    srcw = xs_src_d[b * cfg.BANK:(b + 1) * cfg.BANK, 0:C]
                elif b < HB:
                    srcw = xs_fullA[b * cfg.BANK:(b + 1) * cfg.BANK, 0:C]
                else:
                    srcw = xs_fullB[(b - HB) * cfg.BANK:
                                    (b - HB + 1) * cfg.BANK, 0:C]
                while g0 < gs:
                    k = min(MAXN // P, gs - g0)
                    n = k * P
                    o = doff + g0 * P
                    q = (queue_map[len(gather_insts)] if queue_map else 0)
                    gi = _dma_gather_small(
                        nc.gpsimd,
                        out_ap=tmph[:, gb + g0:gb + g0 + k, :],
                        in_ap=srcw,
                        idxs_ap=gtab[:, o // 16:(o + n) // 16],
                        num_idxs=n, elem_size=C, elem_step=ROWP,
                        queue_num=q)
                    gather_insts.append(gi)

                    g0 += k

            for layer in range(2):
                tiles = {}
                if layer == 1:
                    # bridge the AG-B latency: A-half gathers of the first
                    # LOOKAHEAD sections depend only on AG-A; the AG-B
                    # collective is emitted mid-way so the Pool stream covers
                    # both its dispatch-wait and its transfer time
                    nla = min(LOOKAHEAD, NSEC)
                    for s in range(nla):
                        t = tp.tile([P, GMAX, C], F16, tag="tmph")
                        for b in range(HB):
                            emit_bank(1, s, b, t)
                        tiles[s] = t
                        if s == min(1, nla - 1):
                            nc.gpsimd.collective_compute(
                                "AllGather", mybir.AluOpType.bypass,
                                replica_groups=RG, ins=[ag_b[:]],
                                outs=[xs_fullB[:, :]])
                for s in range(NSEC):
                    gsec = int(sec_groups[s])
                    if s in tiles:
                        tmph = tiles.pop(s)
                        banks = range(HB, NB)
                    else:
                        tmph = tp.tile([P, GMAX, C], F16, tag="tmph")
                        banks = range(NB)
                    for b in banks:
                        emit_bank(layer, s, b, tmph)
                    if layer == 0 and s == min(cfg.SPLIT_SEC + 1, NSEC - 1):
                        nc.gpsimd.collective_compute(
                            "AllGather", mybir.AluOpType.bypass,
                            replica_groups=RG, ins=[ag_a[:]],
                            outs=[xs_fullA[:, :]])
                    # matmuls, chunk-major; W blocks generated on demand
                    clo, chi = int(meta["sec_col_lo"][s]), int(meta["sec_col_lo"][s + 1])
                    psb = pp.tile([P, 512], F32, tag="ps")
                    Wt = None
                    wlo = -1
                    # group columns by local chunk for start/stop flags
                    from collections import defaultdict
                    bych = defaultdict(list)
                    for ci in range(clo, chi):
                        _, lc, b, g = cols[ci]
                        bych[lc].append((ci, b, g))
                    for lc in range(K):
                        lst = bych.get(lc, [])
                        for j, (ci, b, g) in enumerate(lst):
                            blk = ci // WBLK
                            if blk != wlo:
                                wlo = blk
                                Wt = wp.tile([P, WBLK, P], F16, tag="W")
                                c0 = blk * WBLK
                                c1 = min(c0 + WBLK, NCOL)
                                lb = ltab[:, c0:c1].rearrange(
                                    "p (g o) -> p g o", o=1).to_broadcast(
                                        [P, c1 - c0, P])
                                ib = iota[:].rearrange(
                                    "p (o j) -> p o j", o=1).to_broadcast(
                                        [P, c1 - c0, P])
                                nc.vector.tensor_tensor(
                                    out=Wt[:, 0:c1 - c0, :], in0=lb, in1=ib,
                                    op=mybir.AluOpType.is_equal)
                            gidx = int(grp_base[s, b]) + g
                            nc.tensor.matmul(
                                psb[:, lc * C:(lc + 1) * C],
                                Wt[:, ci - wlo * WBLK, :],
                                tmph[:, gidx, :],
                                start=(j == 0), stop=(j == len(lst) - 1))
                    # epilogue: prop = ps*scale + xs0 (layer1 -> fp16 table)
                    ch_lo = s * K
                    pr = (prop16[:, ch_lo:ch_lo + K, 0:C] if layer == 0
                          else prop[:, ch_lo:ch_lo + K, :])
                    scale = s1 if layer == 0 else s3
                    sb = scale[:, ch_lo:ch_lo + K].rearrange(
                        "p (k o) -> p k o", o=1).to_broadcast([P, K, C])
                    psv = psb[:, 0:K * C].rearrange("p (k c) -> p k c", c=C)
                    nc.vector.tensor_tensor(out=pr, in0=psv, in1=sb,
                                            op=mybir.AluOpType.mult)
                    nc.vector.tensor_tensor(out=pr, in0=pr,
                                            in1=xs0[:, ch_lo:ch_lo + K, :],
                                            op=mybir.AluOpType.add)
                    if layer == 1:
                        nc.sync.dma_start(
                            out_d[:, ch_lo * C:(ch_lo + K) * C].rearrange(
                                "p (k c) -> p k c", c=C), pr)
                    if layer == 0 and s == cfg.SPLIT_SEC:
                        nc.sync.dma_start(
                            ag_a[:].rearrange("(l k) c -> l k c", l=P),
                            prop16[:, 0:CH0, :])
                    if layer == 0 and s == NSEC - 1:
                        nc.sync.dma_start(
                            ag_b[:].rearrange("(l k) c -> l k c", l=P),
                            prop16[:, CH0:NCH, :])
                if layer == 0:
                    # prepare layer-2 epilogue adds: xs0 <- alpha*x
                    nc.sync.dma_start(
                        xs0[:], xpc_d[:].rearrange("p (k c) -> p k c", c=C))
                    nc.vector.tensor_scalar_mul(xs0[:], xs0[:], ALPHA)

    if compile_:
        nc.compile()
    return nc, gather_insts


def _scheduled_lanes(gather_insts):
    """DMASW lane index (0-7) of each gather, post Tile scheduling."""
    lanes = []
    for gi in gather_insts:
        inst = getattr(gi, "ins", gi)
        proc = getattr(inst, "bass_scheduled_proc", None)
        assert proc is not None, "gather missing bass_scheduled_proc"
        lanes.append(proc)
    base = min(lanes)
    return [l - base for l in lanes]


def _build_two_pass(cfg, meta):
    """Pass 1: schedule with queue 0 to learn DMASW lane assignment; pass 2:
    rebuild with queue = lane % 4 so the SWDGE sem/queue binding is
    consistent under Tile's (reordered) schedule."""
    nc1, gis1 = _build_program(cfg, meta, queue_map=None, compile_=False)
    lanes = _scheduled_lanes(gis1)
    qmap = [l % 4 for l in lanes]
    nc2, gis2 = _build_program(cfg, meta, queue_map=qmap, compile_=True)
    lanes2 = _scheduled_lanes(gis2)
    if lanes2 != lanes:
        # schedule shifted; one more fixpoint iteration
        qmap = [l % 4 for l in lanes2]
        nc2, gis2 = _build_program(cfg, meta, queue_map=qmap, compile_=True)
        lanes3 = _scheduled_lanes(gis2)
        assert [l % 4 for l in lanes3] == qmap, "queue/lane fixpoint failed"
    return nc2


def _make_in_maps(cfg, meta, gtab, lane_tbl, x):
    C = cfg.C
    xp = np.zeros((cfg.N_PAD, C), np.float32)
    xp[:cfg.N_NODES] = np.asarray(x, np.float32)
    dis = meta["dis"]
    xs_src = np.zeros((cfg.N_PAD, ROWP), np.float16)
    xs_src[meta["gpos"], :C] = (dis[:, None] * xp).astype(np.float16)
    gtw = _wrap16(gtab)
    iota = np.ascontiguousarray(
        np.tile(np.arange(P, dtype=np.float16), (P, 1)))
    in_maps = []
    for c in range(cfg.N_CORES):
        xl = xp[c * cfg.LOCAL:(c + 1) * cfg.LOCAL]
        dl = dis[c * cfg.LOCAL:(c + 1) * cfg.LOCAL]
        x_pc = np.ascontiguousarray(
            xl.reshape(cfg.NCH, P, C).transpose(1, 0, 2).reshape(P, cfg.NCH * C))
        dis_pc = np.ascontiguousarray(dl.reshape(cfg.NCH, P).T)
        in_maps.append({
            "x_pc": x_pc, "dis_pc": dis_pc,
            "gtab": np.ascontiguousarray(gtw[c]),
            "ltab": np.ascontiguousarray(lane_tbl[c]),
            "iota": iota,
            "xs_src": xs_src,
        })
    return in_maps


def _unpermute(cfg, outs_pc):
    res = np.zeros((cfg.N_PAD, cfg.C), np.float32)
    for c in range(cfg.N_CORES):
        xl = outs_pc[c].reshape(P, cfg.NCH, cfg.C).transpose(1, 0, 2).reshape(
            cfg.LOCAL, cfg.C)
        res[c * cfg.LOCAL:(c + 1) * cfg.LOCAL] = xl
    return res[:cfg.N_NODES]


_CACHE = {}


def _get_compiled(cfg, edge_index):
    key = hash(np.asarray(edge_index, np.int64).tobytes())
    if key not in _CACHE:
        meta, gtab, lane_tbl = _preprocess(cfg, np.asarray(edge_index, np.int64))
        nc = _build_two_pass(cfg, meta)
        _CACHE[key] = (meta, gtab, lane_tbl, nc)
    return _CACHE[key]


def kernel(x, edge_index, stack_weights=None, _trace=False, _tmpdir=None):
    cfg = CFG_FULL
    x = np.asarray(x, np.float32)
    meta, gtab, lane_tbl, nc = _get_compiled(cfg, edge_index)
    in_maps = _make_in_maps(cfg, meta, gtab, lane_tbl, x)
    res = bass_utils.run_bass_kernel_spmd(
        nc, in_maps, core_ids=list(range(cfg.N_CORES)), trace=_trace,
        tmpdir=_tmpdir)
    outs = [res.results[c]["out"] for c in range(cfg.N_CORES)]
    full = _unpermute(cfg, outs)
    kernel.last_result = res
    return full
